# revision 1
# baseline (speedup 1.0000x reference)
"""Trainium2 Bass kernel for a diagonal SSM layer (B=8, S=4096, D=1024, N=4).

Math per batch row (one NeuronCore per batch row, data-parallel over batch):
    u  = x @ B_w.T + B_b                    (S, N)
    h_t = lam * h_{t-1} + u_t               (sequential scan, lam = sigmoid(log_lambda))
    y  = h @ C_w.T + C_b                    (S, D)
    out = LayerNorm(x + y) * ln_w + ln_b

Device mapping (DEFAULT_CFG; ~114 us/core cost-model vs ~94 us memory
roofline; HW-validated rel err 8.9e-7, i.e. exact-class):
  - x streams in per 128-step chunk (natural layout: steps on partitions),
    one 512 KiB DMA per chunk, 12 chunks resident.
  - Per chunk: PE transposes the 8 128x128 subtiles -> PSUM -> ACT copies
    to SBUF (x.T, d on partitions); 8 cheap N=4 fp32 matmuls (x.T tiles
    stationary) accumulate u natural (128,4); a tiny PE transpose flips it
    to u.T (4,128), and the PSUM->SBUF copy rides B_b in via the ACT bias.
  - The sequential scan h = lam*h + u runs on the DVE with
    tensor_tensor_scan (a native per-partition prefix scan), one 128-step
    segment per chunk, chained via initial=prev[:, -1:]. Handles per-state
    lambdas natively (lam is a per-partition operand).
  - y + C_b: rank-5 matmuls per 512-wide half (h rows 0..3 plus a ones
    row carrying C_b) in float32r hi+lo form (3 cheap f32r matmuls per
    half reconstruct full fp32 precision, TF32x3-style; hi/lo splits run
    on the idle GPSIMD); residual x added on the DVE (tensor_tensor).
  - LayerNorm: bn_stats/bn_aggr (DVE), rstd via ACT sqrt + DVE reciprocal,
    tiny scalar ops on the otherwise-idle GPSIMD, apply via one ACT
    activation with per-partition scale/bias, then a 512 KiB DMA out.
  - Emission is software-pipelined: phase A of chunk c+1 is emitted before
    phase B of chunk c (cfg.skew).
Cfg also exposes float32r variants (~10% faster end-to-end in the cost
model but ~1e-4 rel err - float32r is a reduced-precision PE format) and a
_split_excess_waits post-pass that works around this container's walrus
accepting at most one sem-wait per instruction.
"""

import numpy as np

import concourse.bass as bass
import concourse.mybir as mybir
from concourse import tile
from concourse.bass_utils import run_bass_kernel_spmd

NB, S, D, NS = 8, 4096, 1024, 4
L = 128                 # chunk length (timesteps per partition block)
NCH = S // L            # 32 chunks
GRP = 4                 # chunks per group
NG = NCH // GRP         # 8 groups
SEG = GRP * L           # 512 timesteps per group segment
LN_EPS = 1e-5
F32 = mybir.dt.float32
F32R = mybir.dt.float32r
AF = mybir.ActivationFunctionType
ALU = mybir.AluOpType


def _split_excess_waits(nc, max_waits=1):
    """This toolchain's walrus accepts at most one sem-wait per instruction.
    Move extra waits onto preceding same-engine NOPs (engines are in-order,
    so an earlier wait on the same engine is equivalent)."""
    moved = 0
    for f in nc.m.functions:
        for bb in f.blocks:
            out = []
            for inst in bb.instructions:
                si = inst.sync_info
                if si is not None and len(si.on_wait) > max_waits:
                    waits = list(si.on_wait)
                    for w in waits[:-max_waits]:
                        nop = mybir.InstNoOp(
                            name=f"WSPL-{nc.next_id()}", ins=[], outs=[])
                        nop.engine = inst.engine
                        nop.sync_info = mybir.SyncInfo(on_wait=[w], on_update=[])
                        out.append(nop)
                        moved += 1
                    inst.sync_info = mybir.SyncInfo(
                        on_wait=waits[-max_waits:], on_update=list(si.on_update))
                if si is not None and len(si.on_update) > 1:
                    raise RuntimeError(
                        f"instruction {inst.name} has {len(si.on_update)} updates")
                out.append(inst)
            bb.instructions = out
    return moved


class Cfg:
    def __init__(self, u_f32r=False, y_f32r=False, tr_f32r=False,
                 resid_pe=False, resid_f32r=False, ln_affine=False,
                 xbufs=32, zbufs=3, obufs=3, xtbufs=3, utbufs=3,
                 ptrbufs=2, pubufs=2, pzbufs=2, cb_dve_mod=-1,
                 groups=(4,) * 8, pair_dma=False, u_natural=False,
                 putbufs=1, tiny_gpsimd=False, skew=0,
                 out_dma_act=False, usb_act=False, scan_gpsimd=False,
                 bn3d=False, ptrbig=False, halfz=False, y_f32r3=False,
                 scan_psum=False, sqrtb=False, stbufs=6):
        self.u_f32r = u_f32r          # float32r for the u-projection matmuls
        self.y_f32r = y_f32r          # float32r for the y matmuls
        self.tr_f32r = tr_f32r        # float32r transpose mode
        self.resid_pe = resid_pe      # residual add via identity matmul on PE
        self.resid_f32r = resid_f32r  # float32r for the identity matmul
        self.ln_affine = ln_affine    # emit ln_w/ln_b application
        self.xbufs = xbufs
        self.zbufs = zbufs
        self.obufs = obufs
        self.xtbufs = xtbufs
        self.utbufs = utbufs
        self.ptrbufs = ptrbufs
        self.pubufs = pubufs
        self.pzbufs = pzbufs
        self.cb_dve_mod = cb_dve_mod  # j%4 == this -> transpose copyback on DVE
        self.groups = tuple(groups)   # chunks per scan segment, sums to 32
        assert sum(self.groups) == NCH
        self.pair_dma = pair_dma      # 2 chunks (1 MiB) per in/out DMA
        # u matmul orientation: stationary x.T tiles producing u natural
        # (cheap N=4 fp32 matmuls), then a tiny transpose to u.T.
        self.u_natural = u_natural
        self.putbufs = putbufs
        self.tiny_gpsimd = tiny_gpsimd
        self.skew = skew
        self.out_dma_act = out_dma_act
        self.usb_act = usb_act
        self.scan_gpsimd = scan_gpsimd
        self.bn3d = bn3d
        self.ptrbig = ptrbig
        self.halfz = halfz
        self.y_f32r3 = y_f32r3    # y matmul as f32r hi+lo (3 matmuls, ~fp32)
        self.scan_psum = scan_psum  # scan reads u.T straight from PSUM;
                                    # B_b enters via a K=1 matmul
        self.sqrtb = sqrtb          # batch rstd (sqrt+recip) over N chunks
        self.stbufs = stbufs
        self.key = (u_f32r, y_f32r, tr_f32r, resid_pe, resid_f32r, ln_affine,
                    xbufs, zbufs, obufs, xtbufs, utbufs,
                    ptrbufs, pubufs, pzbufs, cb_dve_mod,
                    self.groups, pair_dma, u_natural, putbufs,
                    tiny_gpsimd, skew, out_dma_act, usb_act, scan_gpsimd,
                    bn3d, ptrbig, halfz, y_f32r3, scan_psum, sqrtb,
                    stbufs)


def _r(ap, flag):
    return ap.bitcast(F32R) if flag else ap


def build_nc(cfg: Cfg):
    nc = bass.Bass("TRN2", target_bir_lowering=False, debug=False)

    # float32r is a reduced-precision PE format: any SBUF operand of an
    # f32r matmul must itself be written as f32r (verifier-enforced), so
    # the dtype is threaded through tiles and DRAM decls rather than cast.
    if cfg.tr_f32r:
        assert cfg.u_f32r, "f32r transposes feed the u matmul; enable u_f32r"
    if cfg.u_natural:
        assert not (cfg.u_f32r or cfg.tr_f32r)
    XDT = F32R if cfg.tr_f32r else F32
    UDT = F32R if cfg.u_f32r else F32
    YDT = F32R if cfg.y_f32r else F32

    x_d = nc.dram_tensor("x", [S, D], XDT, kind="ExternalInput")
    bwt_d = nc.dram_tensor("bwt", [128, 8 * NS], UDT, kind="ExternalInput")
    lamb_d = nc.dram_tensor("lamb", [NS, SEG], F32, kind="ExternalInput")
    bb_d = nc.dram_tensor("bb", [NS, 1], F32, kind="ExternalInput")
    if cfg.scan_psum:
        bbr_d = nc.dram_tensor("bbr", [1, NS], F32, kind="ExternalInput")
    cwa_d = nc.dram_tensor("cwa", [NS + 1, D], YDT, kind="ExternalInput")
    idn_d = nc.dram_tensor("idn", [128, 128], XDT, kind="ExternalInput")
    ones_d = nc.dram_tensor("ones", [1, S], YDT, kind="ExternalInput")
    out_d = nc.dram_tensor("out", [S, D], F32, kind="ExternalOutput")
    if cfg.ln_affine:
        lnw_d = nc.dram_tensor("lnw", [128, D], F32, kind="ExternalInput")
        lnb_d = nc.dram_tensor("lnb", [128, D], F32, kind="ExternalInput")

    with tile.TileContext(nc) as tc:
        with (
            tc.tile_pool(name="consts", bufs=1) as cpool,
            tc.tile_pool(name="xin", bufs=cfg.xbufs) as xpool,
            tc.tile_pool(name="xT", bufs=cfg.xtbufs) as xtp,
            tc.tile_pool(name="uT", bufs=cfg.utbufs) as utp,
            tc.tile_pool(name="hseq", bufs=1) as hsp,
            tc.tile_pool(name="zs", bufs=cfg.zbufs) as zsp,
            tc.tile_pool(name="stats", bufs=cfg.stbufs) as stp,
            tc.tile_pool(name="outb", bufs=cfg.obufs) as obp,
            tc.tile_pool(name="ps_tr", bufs=cfg.ptrbufs, space="PSUM") as ptr,
            tc.tile_pool(name="ps_u", bufs=cfg.pubufs, space="PSUM") as pup,
            tc.tile_pool(name="ps_ut", bufs=cfg.putbufs, space="PSUM") as put_p,
            tc.tile_pool(name="ps_z", bufs=cfg.pzbufs, space="PSUM") as pzp,
        ):
            bwt = cpool.tile([128, 8 * NS], UDT)
            nc.sync.dma_start(bwt[:], bwt_d[:])
            lamb = cpool.tile([NS, SEG], F32)
            nc.sync.dma_start(lamb[:], lamb_d[:])
            bb = cpool.tile([NS, 1], F32)
            nc.sync.dma_start(bb[:], bb_d[:])
            if cfg.scan_psum:
                bbr = cpool.tile([1, NS], F32)
                nc.sync.dma_start(bbr[:], bbr_d[:])
                onesr = cpool.tile([1, S], F32)
                nc.sync.dma_start(onesr[:], ones_d[:])
            cwa = cpool.tile([NS + 1, D], YDT)
            nc.sync.dma_start(cwa[:], cwa_d[:])
            if cfg.y_f32r3:
                cwah = cpool.tile([NS + 1, D], F32R)
                nc.scalar.copy(cwah[:], cwa[:])
                cwal = cpool.tile([NS + 1, D], F32R)
                nc.vector.tensor_tensor(cwal[:], cwa[:], cwah[:], ALU.subtract)
            idn = cpool.tile([128, 128], XDT)
            nc.sync.dma_start(idn[:], idn_d[:])
            if cfg.ln_affine:
                lnw = cpool.tile([128, D], F32)
                nc.sync.dma_start(lnw[:], lnw_d[:])
                lnb = cpool.tile([128, D], F32)
                nc.sync.dma_start(lnb[:], lnb_d[:])

            # h sequence (rows 0..3) plus a constant ones row (carries C_b
            # through the rank-5 y matmul). Static tile; row 4 loaded once.
            hs = hsp.tile([NS + 1, S], YDT)
            nc.sync.dma_start(hs[NS:NS + 1, :], ones_d[:])

            # x chunk tiles: one chunk (512 KiB) or two (1 MiB) per DMA
            if cfg.pair_dma:
                xpair = []
                for p in range(NCH // 2):
                    t = xpool.tile([128, 2 * D], XDT, tag="xc")
                    nc.sync.dma_start(
                        t[:].rearrange("p (c d) -> p c d", c=2),
                        x_d[p * 2 * L:(p + 1) * 2 * L, :].rearrange(
                            "(c p) d -> p c d", p=128))
                    xpair.append(t)

                def xap(c):
                    return xpair[c // 2][:, (c % 2) * D:(c % 2 + 1) * D]
            else:
                xc = []
                for c in range(NCH):
                    t = xpool.tile([128, D], XDT, tag="xc")
                    nc.sync.dma_start(t[:], x_d[c * L:(c + 1) * L, :])
                    xc.append(t)

                def xap(c):
                    return xc[c][:]

            ob_state = [None]  # paired output staging
            sq_state = {}      # sqrt batch state (cfg.sqrtb)
            def phase_a(c0g, gsz):
                glen = gsz * L
                # ---- Phase A: transpose gsz chunks, project to u.T ----
                if cfg.u_natural:
                    uT = utp.tile([NS, SEG], F32, tag="uT")
                    for ci in range(gsz):
                        c = c0g + ci
                        xt = xtp.tile([128, 8 * L], F32, tag="xt")
                        if cfg.ptrbig:
                            pt = ptr.tile([128, 8 * L], XDT, tag="pt")
                            for j in range(8):
                                nc.tensor.transpose(
                                    pt[:, j * L:(j + 1) * L],
                                    xap(c)[:, j * 128:(j + 1) * 128],
                                    idn[:])
                            nc.scalar.copy(xt[:], pt[:])
                        else:
                            for jh in range(2):
                                pt = ptr.tile([128, SEG], XDT, tag="pt")
                                for j4 in range(4):
                                    j = jh * 4 + j4
                                    nc.tensor.transpose(
                                        pt[:, j4 * L:(j4 + 1) * L],
                                        xap(c)[:, j * 128:(j + 1) * 128],
                                        idn[:])
                                nc.scalar.copy(
                                    xt[:, jh * SEG:(jh + 1) * SEG], pt[:])
                        pnat = pup.tile([128, NS], F32, tag="pu")
                        for j in range(8):
                            nc.tensor.matmul(
                                pnat[:],
                                lhsT=xt[:, j * L:(j + 1) * L],
                                rhs=bwt[:, j * NS:(j + 1) * NS],
                                start=(j == 0), stop=(j == 7))
                        usb = stp.tile([128, NS], F32, tag="usb")
                        if cfg.usb_act:
                            nc.scalar.copy(usb[:], pnat[:])
                        else:
                            nc.vector.tensor_copy(usb[:], pnat[:])
                        putt = put_p.tile([NS, L], F32, tag="put")
                        if cfg.scan_psum:
                            nc.tensor.matmul(putt[:], lhsT=usb[:], rhs=idn[:],
                                             is_transpose=True,
                                             start=True, stop=False)
                            nc.tensor.matmul(putt[:], lhsT=bbr[:],
                                             rhs=onesr[:, 0:L],
                                             start=False, stop=True)
                            last_putt = putt
                        else:
                            nc.tensor.transpose(putt[:], usb[:], idn[:])
                            nc.scalar.activation(
                                uT[:, ci * L:(ci + 1) * L], putt[:],
                                AF.Identity, bias=bb[:], scale=1.0)
                    t0 = c0g * L
                    seg = slice(t0, t0 + glen)
                    init = 0.0 if c0g == 0 else hs[0:NS, t0 - 1:t0]
                    scan_eng = nc.gpsimd if cfg.scan_gpsimd else nc.vector
                    if cfg.scan_psum:
                        assert glen == L, "scan_psum needs unit groups"
                        scan_eng.tensor_tensor_scan(
                            hs[0:NS, seg], lamb[:, :glen], last_putt[:],
                            initial=init, op0=ALU.mult, op1=ALU.add)
                    else:
                        scan_eng.tensor_tensor_scan(
                            hs[0:NS, seg], lamb[:, :glen], uT[:, :glen],
                            initial=init, op0=ALU.mult, op1=ALU.add)
                    if cfg.y_f32r3 and glen == L:
                        emit_split(c0g)
                else:
                    pu = pup.tile([NS, SEG], F32, tag="pu")
                    for j in range(8):
                        pt = ptr.tile([128, SEG], XDT, tag="pt")
                        for ci in range(gsz):
                            c = c0g + ci
                            nc.tensor.transpose(
                                pt[:, ci * L:(ci + 1) * L],
                                xap(c)[:, j * 128:(j + 1) * 128],
                                idn[:],
                            )
                        xt = xtp.tile([128, SEG], UDT, tag="xt")
                        if j % 4 == cfg.cb_dve_mod:
                            nc.vector.tensor_copy(xt[:, :glen], pt[:, :glen])
                        else:
                            nc.scalar.copy(xt[:, :glen], pt[:, :glen])
                        nc.tensor.matmul(
                            pu[:, :glen],
                            lhsT=bwt[:, j * NS:(j + 1) * NS],
                            rhs=xt[:, :glen],
                            start=(j == 0), stop=(j == 7),
                        )
                    # PSUM -> SBUF with B_b folded in via the ACT bias
                    uT = utp.tile([NS, SEG], F32, tag="uT")
                    nc.scalar.activation(uT[:, :glen], pu[:, :glen],
                                         AF.Identity, bias=bb[:], scale=1.0)

                    # -- sequential scan: h = lam*h + u, chained across groups
                    t0 = c0g * L
                    seg = slice(t0, t0 + glen)
                    init = 0.0 if c0g == 0 else hs[0:NS, t0 - 1:t0]
                    scan_eng = nc.gpsimd if cfg.scan_gpsimd else nc.vector
                    scan_eng.tensor_tensor_scan(
                        hs[0:NS, seg], lamb[:, :glen], uT[:, :glen],
                        initial=init, op0=ALU.mult, op1=ALU.add)

            hs_split = {}  # c -> (hsh, hsl), emitted in phase A under skew

            def emit_split(c):
                lhs_y = hs[:, c * L:(c + 1) * L]
                hsh = stp.tile([NS + 1, L], F32R, tag="hsh")
                nc.gpsimd.tensor_copy(hsh[:], lhs_y)
                hsl = stp.tile([NS + 1, L], F32R, tag="hsl")
                nc.gpsimd.tensor_tensor(hsl[:], lhs_y, hsh[:], ALU.subtract)
                hs_split[c] = (hsh, hsl)

            def phase_b(c):
                z = pzp.tile([128, D], F32, tag="z")
                lhs_y = hs[:, c * L:(c + 1) * L]
                if cfg.y_f32r3:
                    hsh, hsl = (hs_split.pop(c) if c in hs_split
                                else (None, None))
                    if hsh is None:
                        emit_split(c)
                        hsh, hsl = hs_split.pop(c)
                    for h in range(2):
                        cols = slice(h * 512, (h + 1) * 512)
                        nc.tensor.matmul(z[:, cols], lhsT=hsh[:],
                                         rhs=cwah[:, cols],
                                         start=True, stop=False)
                        nc.tensor.matmul(z[:, cols], lhsT=hsh[:],
                                         rhs=cwal[:, cols],
                                         start=False, stop=False)
                        nc.tensor.matmul(z[:, cols], lhsT=hsl[:],
                                         rhs=cwah[:, cols],
                                         start=False, stop=True)
                for h in range(2) if not cfg.y_f32r3 else ():
                    cols = slice(h * 512, (h + 1) * 512)
                    if cfg.resid_pe:
                        nc.tensor.matmul(
                            z[:, cols],
                            lhsT=_r(idn[:], cfg.resid_f32r),
                            rhs=_r(xap(c)[:, cols], cfg.resid_f32r),
                            start=True, stop=False)
                        nc.tensor.matmul(
                            z[:, cols],
                            lhsT=lhs_y,
                            rhs=cwa[:, cols],
                            start=False, stop=True)
                    else:
                        nc.tensor.matmul(
                            z[:, cols],
                            lhsT=lhs_y,
                            rhs=cwa[:, cols],
                            start=True, stop=True)
                if cfg.resid_pe:
                    zsrc = z
                else:
                    zsb = zsp.tile([128, D], F32, tag="zsb")
                    xin = xap(c).bitcast(F32) if cfg.tr_f32r else xap(c)
                    if cfg.halfz:
                        nc.vector.tensor_tensor(
                            zsb[:, 0:512], xin[:, 0:512], z[:, 0:512], ALU.add)
                        nc.vector.tensor_tensor(
                            zsb[:, 512:1024], xin[:, 512:1024], z[:, 512:1024],
                            ALU.add)
                    else:
                        nc.vector.tensor_tensor(zsb[:], xin, z[:], ALU.add)
                    zsrc = zsb

                st = stp.tile([128, 12], F32, tag="st")
                nc.vector.bn_stats(st[:, 0:6], zsrc[:, 0:512])
                nc.vector.bn_stats(st[:, 6:12], zsrc[:, 512:1024])
                mv = stp.tile([128, 2], F32, tag="mv")
                nc.vector.bn_aggr(mv[:], st[:])
                tiny = nc.gpsimd if cfg.tiny_gpsimd else nc.vector
                if cfg.sqrtb:
                    assert not cfg.resid_pe
                    n = cfg.sqrtb
                    qi = c % n
                    if qi == 0:
                        vb = stp.tile([128, n], F32, tag="vb")
                        rb = stp.tile([128, n], F32, tag="rb")
                        sq_state.clear()
                        sq_state.update(vb=vb, rb=rb, items=[])
                    tiny.tensor_scalar_add(
                        sq_state["vb"][:, qi:qi + 1], mv[:, 1:2], LN_EPS)
                    sq_state["items"].append((c, mv, zsrc))
                    if qi == n - 1:
                        sb = stp.tile([128, n], F32, tag="sb")
                        nc.scalar.sqrt(sb[:], sq_state["vb"][:])
                        nc.vector.reciprocal(sq_state["rb"][:], sb[:])
                        for (c2, mv2, zsrc2) in sq_state["items"]:
                            _apply_tail(c2, mv2, zsrc2,
                                        sq_state["rb"][:, c2 % n:c2 % n + 1])
                    return
                veps = stp.tile([128, 1], F32, tag="veps")
                tiny.tensor_scalar_add(veps[:], mv[:, 1:2], LN_EPS)
                std = stp.tile([128, 1], F32, tag="std")
                nc.scalar.sqrt(std[:], veps[:])
                rstd = stp.tile([128, 1], F32, tag="rstd")
                nc.vector.reciprocal(rstd[:], std[:])
                _apply_tail(c, mv, zsrc, rstd[:])

            def _apply_tail(c, mv, zsrc, rstd_ap):
                tiny = nc.gpsimd if cfg.tiny_gpsimd else nc.vector
                nmr = stp.tile([128, 1], F32, tag="nmr")
                tiny.tensor_scalar(
                    nmr[:], mv[:, 0:1], rstd_ap, -1.0,
                    ALU.mult, ALU.mult)

                if cfg.pair_dma:
                    if c % 2 == 0:
                        ob_new = obp.tile([128, 2 * D], F32, tag="ob")
                        ob_state[0] = ob_new
                    ob_t = ob_state[0]
                    oap = ob_t[:, (c % 2) * D:(c % 2 + 1) * D]
                else:
                    ob_t = obp.tile([128, D], F32, tag="ob")
                    oap = ob_t[:]
                nc.scalar.activation(oap, zsrc[:], AF.Identity,
                                     bias=nmr[:], scale=rstd_ap)
                if cfg.ln_affine:
                    nc.vector.tensor_tensor(oap, oap, lnw[:], ALU.mult)
                    nc.vector.tensor_tensor(oap, oap, lnb[:], ALU.add)
                if cfg.pair_dma:
                    if c % 2 == 1:
                        p = c // 2
                        nc.sync.dma_start(
                            out_d[p * 2 * L:(p + 1) * 2 * L, :].rearrange(
                                "(c p) d -> p c d", p=128),
                            ob_t[:].rearrange("p (c d) -> p c d", c=2))
                else:
                    nc.sync.dma_start(out_d[c * L:(c + 1) * L, :], ob_t[:])

            # driver: emit phase A `skew` chunks ahead of phase B
            starts = []
            c0g = 0
            for gsz in cfg.groups:
                starts.append((c0g, gsz))
                c0g += gsz
            if cfg.skew == 0:
                for c0g, gsz in starts:
                    phase_a(c0g, gsz)
                    for ci in range(gsz):
                        phase_b(c0g + ci)
            else:
                assert all(g == 1 for g in cfg.groups), "skew needs unit groups"
                for c in range(NCH + cfg.skew):
                    if c < NCH:
                        phase_a(c, 1)
                    if c >= cfg.skew:
                        phase_b(c - cfg.skew)

    _split_excess_waits(nc)
    return nc


_NC_CACHE = {}


def _get_nc(cfg: Cfg):
    if cfg.key not in _NC_CACHE:
        _NC_CACHE[cfg.key] = build_nc(cfg)
    return _NC_CACHE[cfg.key]


# Best known-exact configuration (hardware-validated, rel err ~8.9e-7 vs the
# fp32 jax reference; cost-model estimate ~114 us/core vs ~94 us memory
# roofline). y_f32r3 keeps fp32-class precision; the single-pass u_f32r /
# y_f32r / tr_f32r variants are faster still but cost ~1e-4 rel err.
DEFAULT_CFG = Cfg(u_natural=True, groups=(1,) * NCH,
                  xbufs=12, zbufs=8, obufs=8, xtbufs=6, utbufs=4,
                  pubufs=2, putbufs=2, pzbufs=1, tiny_gpsimd=True, skew=2,
                  usb_act=True, y_f32r3=True)


def make_inputs(x, log_lambda, B_w, B_b, C_w, C_b, ln_w, ln_b, cfg):
    lam = (1.0 / (1.0 + np.exp(-np.float64(log_lambda)))).astype(np.float32)
    # bwt[p, 4j+n] = B_w[n, 128j+p]
    bwt = np.ascontiguousarray(
        np.transpose(np.asarray(B_w, np.float32).T.reshape(8, 128, NS),
                     (1, 0, 2)).reshape(128, 8 * NS))
    lamb = np.ascontiguousarray(np.tile(lam[:, None], (1, SEG)))
    bb = np.ascontiguousarray(np.asarray(B_b, np.float32)[:, None])
    cwa = np.ascontiguousarray(
        np.concatenate([np.asarray(C_w, np.float32).T,
                        np.asarray(C_b, np.float32)[None, :]], 0))
    idn = np.eye(128, dtype=np.float32)
    ones = np.ones((1, S), np.float32)
    shared = {"bwt": bwt, "lamb": lamb, "bb": bb, "cwa": cwa, "idn": idn,
              "ones": ones,
              "bbr": np.ascontiguousarray(np.asarray(B_b, np.float32)[None, :])}
    if cfg.ln_affine:
        shared["lnw"] = np.ascontiguousarray(
            np.tile(np.asarray(ln_w, np.float32)[None, :], (128, 1)))
        shared["lnb"] = np.ascontiguousarray(
            np.tile(np.asarray(ln_b, np.float32)[None, :], (128, 1)))
    x = np.asarray(x, np.float32)
    return [dict(shared, x=np.ascontiguousarray(x[b])) for b in range(NB)]


def run(inputs, cfg=None, **spmd_kwargs):
    cfg = cfg or DEFAULT_CFG
    ln_w = np.asarray(inputs["ln_w"], np.float32)
    ln_b = np.asarray(inputs["ln_b"], np.float32)
    affine = not (np.allclose(ln_w, 1.0) and np.allclose(ln_b, 0.0))
    if affine != cfg.ln_affine:
        import copy as _copy
        cfg = _copy.copy(cfg)
        cfg.ln_affine = affine
        cfg.key = cfg.key[:5] + (affine,) + cfg.key[6:]
    nc = _get_nc(cfg)
    in_maps = make_inputs(
        inputs["x"], inputs["log_lambda"], inputs["B_w"], inputs["B_b"],
        inputs["C_w"], inputs["C_b"], ln_w, ln_b, cfg)
    res = run_bass_kernel_spmd(nc, in_maps, core_ids=list(range(NB)),
                               **spmd_kwargs)
    out = np.stack([res.results[b]["out"] for b in range(NB)], 0)
    return out, res


def kernel(**inputs):
    out, _ = run(inputs, DEFAULT_CFG)
    return out

